# revision 1
# baseline (speedup 1.0000x reference)
"""BiLSTM-CRF on 8 Trainium2 NeuronCores (Bass/Tile).

Symmetric SPMD program: core 0 runs the forward LSTM + alpha max-plus scan,
core 1 the backward LSTM + gamma scan (same instruction stream, direction is
pure data: reversed token order / reversed index maps). Cores 2-7 run the
program on zeros. Emissions halves are summed with an AllReduce; both scan
arrays are AllGathered; tags_t = argmax_j(alpha_t[j] + gamma_t[j] - em_t[j]).

LSTM recurrence: fp32r (tf32-class) W-moving matmuls; the input-gate
contribution (xe @ W_ih + b) is accumulated into the same PSUM region as the
recurrent matmuls, two timesteps per 128-token gather tile.
"""
import numpy as np

import concourse.bass as bass
import concourse.tile as tile
from concourse import mybir, bacc
from concourse.bass_utils import run_bass_kernel_spmd
from concourse.masks import make_identity

B, E, H, K, G = 64, 256, 512, 48, 2048
T_FULL = 512
V = 50000
N_CORES = 8
F32 = mybir.dt.float32
F32R = mybir.dt.float32r
I32 = mybir.dt.int32

Q_ORDER = [2, 0, 1, 3]  # gate-quarter PE order: g~ first, o last


def _build_nc(T=T_FULL):
    TT = T * B
    NTILE = TT // 128
    nc = bacc.Bacc("TRN2", target_bir_lowering=False, debug=False,
                   num_devices=N_CORES)

    idx_ap = nc.dram_tensor("idx", [128, NTILE], I32, kind="ExternalInput").ap()
    emb_ap = nc.dram_tensor("emb", [V, E], F32, kind="ExternalInput").ap()
    wih_ap = nc.dram_tensor("wih", [E, G], F32, kind="ExternalInput").ap()
    bias_ap = nc.dram_tensor("bias", [1, G], F32, kind="ExternalInput").ap()
    whh_ap = nc.dram_tensor("whh", [H, G], F32, kind="ExternalInput").ap()
    wout_ap = nc.dram_tensor("wout", [H, K], F32, kind="ExternalInput").ap()
    bout_ap = nc.dram_tensor("bout", [128, K], F32, kind="ExternalInput").ap()
    trep_ap = nc.dram_tensor("trep", [1, K * K], F32, kind="ExternalInput").ap()
    sinit_ap = nc.dram_tensor("sinit", [1, K], F32, kind="ExternalInput").ap()
    emidx_ap = nc.dram_tensor("emidx", [B, T], I32, kind="ExternalInput").ap()
    emwidx_ap = nc.dram_tensor("emwidx", [128, NTILE], I32,
                               kind="ExternalInput").ap()

    tags_ap = nc.dram_tensor("tags", [B, T], I32, kind="ExternalOutput").ap()

    ht_dram = nc.dram_tensor("ht_dram", [T, 128, 256], F32R)
    em_bounce = nc.dram_tensor("em_bounce", [TT, K], F32)
    em_shared = nc.dram_tensor("em_shared", [TT, K], F32, addr_space="Shared")
    scan_loc = nc.dram_tensor("scan_loc", [T * B, K], F32)
    scan_gath = nc.dram_tensor("scan_gath", [N_CORES * T * B, K], F32,
                               addr_space="Shared")

    groups = [list(range(N_CORES))]

    with tile.TileContext(nc) as tc:
        with tc.tile_pool(name="const", bufs=1) as cp:
            idx_sb = cp.tile([128, NTILE], I32)
            nc.sync.dma_start(idx_sb[:], idx_ap[:, :])
            emwidx_sb = cp.tile([128, NTILE], I32)
            nc.sync.dma_start(emwidx_sb[:], emwidx_ap[:, :])
            emidx_sb = cp.tile([B, T], I32)
            nc.sync.dma_start(emidx_sb[:], emidx_ap[:, :])

            ident = cp.tile([128, 128], F32)
            make_identity(nc, ident[:])

            stage_ctx = tc.tile_pool(name="stage", bufs=1)
            sp0 = stage_ctx.__enter__()
            wih_f = sp0.tile([128, 2, G], F32)
            nc.sync.dma_start(wih_f[:], bass.AP(
                tensor=wih_ap.tensor, offset=0,
                ap=[[G, 128], [G * 128, 2], [1, G]]))
            wih_r = cp.tile([128, 2, G], F32R)
            nc.vector.tensor_copy(wih_r[:], wih_f[:])

            whh_f = sp0.tile([128, 4, G], F32)
            nc.sync.dma_start(whh_f[:], bass.AP(
                tensor=whh_ap.tensor, offset=0,
                ap=[[G, 128], [G * 128, 4], [1, G]]))
            whh_r = cp.tile([128, 4, G], F32R)
            nc.vector.tensor_copy(whh_r[:], whh_f[:])

            wout_f = sp0.tile([128, 4, K], F32)
            nc.sync.dma_start(wout_f[:], bass.AP(
                tensor=wout_ap.tensor, offset=0,
                ap=[[K, 128], [K * 128, 4], [1, K]]))
            wout_r = cp.tile([128, 4, K], F32R)
            nc.vector.tensor_copy(wout_r[:], wout_f[:])

            bias_f = sp0.tile([1, G], F32)
            nc.sync.dma_start(bias_f[:], bias_ap[:, :])
            bias_r = cp.tile([1, G], F32R)
            nc.vector.tensor_copy(bias_r[:], bias_f[:])

            ones_f = sp0.tile([1, 128], F32)
            nc.vector.memset(ones_f[:], 1.0)
            ones_r = cp.tile([1, 128], F32R)
            nc.vector.tensor_copy(ones_r[:], ones_f[:])

            bout_sb = cp.tile([128, K], F32)
            nc.sync.dma_start(bout_sb[:], bout_ap[:, :])

            trep_sb = cp.tile([B, K * K], F32)
            nc.sync.dma_start(trep_sb[:], bass.AP(
                tensor=trep_ap.tensor, offset=0, ap=[[0, B], [1, K * K]]))
            sinit_sb = cp.tile([B, K], F32)
            nc.sync.dma_start(sinit_sb[:], bass.AP(
                tensor=sinit_ap.tensor, offset=0, ap=[[0, B], [1, K]]))

            jshift = cp.tile([B, K], F32)
            jshift_i = sp0.tile([B, K], I32)
            nc.gpsimd.iota(jshift_i[:], pattern=[[1, K]], base=0,
                           channel_multiplier=0)
            nc.vector.tensor_copy(jshift[:], jshift_i[:])
            nc.vector.tensor_scalar_sub(jshift[:], jshift[:], 1000.0)

            tags_sb = cp.tile([B, T], I32)
            stage_ctx.__exit__(None, None, None)

            # ---------- fused embed + input-gates + recurrence ----------
            with tc.tile_pool(name="lstm", bufs=2) as lp, \
                 tc.tile_pool(name="psx", bufs=2, space="PSUM") as psx, \
                 tc.tile_pool(name="psg", bufs=1, space="PSUM") as psg, \
                 tc.tile_pool(name="pst", bufs=2, space="PSUM") as pst:

                hT_slot1 = lp.tile([128, 4, 128], F32R, tag="hT", bufs=2,
                                   name="hTinitA")
                nc.vector.memset(hT_slot1[:].bitcast(F32), 0.0)
                hT = lp.tile([128, 4, 128], F32R, tag="hT", bufs=2, name="hT0")
                nc.vector.memset(hT[:].bitcast(F32), 0.0)
                cst = lp.tile([B, H], F32, tag="cst", bufs=2, name="cst0")
                nc.vector.memset(cst[:], 0.0)

                for kt in range(NTILE):
                    xe = lp.tile([128, E], F32, tag="xe", bufs=4,
                                 name=f"xe{kt}")
                    nc.gpsimd.indirect_dma_start(
                        out=xe[:], out_offset=None, in_=emb_ap,
                        in_offset=bass.IndirectOffsetOnAxis(
                            ap=idx_sb[:, kt:kt + 1], axis=0))
                    xet_ps = psx.tile([128, 256], F32, tag="xet",
                                      name=f"xet{kt}")
                    for e in range(2):
                        nc.tensor.transpose(
                            xet_ps[:, e * 128:(e + 1) * 128],
                            xe[:, e * 128:(e + 1) * 128], ident[:])
                    xet = lp.tile([128, 256], F32R, tag="xet_r", bufs=3,
                                  name=f"xetr{kt}")
                    nc.vector.tensor_copy(xet[:], xet_ps[:])

                    gx = psg.tile([128, G], F32, tag="gx", name=f"gx{kt}")
                    for e in range(2):
                        for q in Q_ORDER:
                            sl = slice(q * 512, (q + 1) * 512)
                            nc.tensor.matmul(
                                gx[:, sl], xet[:, e * 128:(e + 1) * 128],
                                wih_r[:, e, sl], start=(e == 0), stop=False,
                                skip_group_check=True)
                    for q in Q_ORDER:
                        sl = slice(q * 512, (q + 1) * 512)
                        nc.tensor.matmul(
                            gx[:, sl], ones_r[0:1, :], bias_r[0:1, sl],
                            start=False, stop=False, skip_group_check=True)

                    for half in range(2):
                        s = 2 * kt + half
                        ro = 64 * half
                        for kk in range(4):
                            for q in Q_ORDER:
                                sl = slice(q * 512, (q + 1) * 512)
                                nc.tensor.matmul(
                                    gx[:, sl], hT[:, kk, :], whh_r[:, kk, sl],
                                    start=False, stop=(kk == 3),
                                    skip_group_check=True)
                        cst_new = lp.tile([B, H], F32, tag="cst", bufs=2,
                                          name=f"cst{s + 1}")
                        hT_new = lp.tile([128, 4, 128], F32R, tag="hT",
                                         bufs=2, name=f"hT{s + 1}")
                        nhalf = (s + 1) % 2
                        tr_ps = pst.tile([128, 256], F32, tag="tr",
                                         name=f"tr{s}")
                        for hf, (co, sz) in enumerate(
                                ((0, 256), (256, 256))):
                            csl = slice(co, co + sz)
                            nch = sz // 128
                            tg = lp.tile([B, sz], F32, tag=f"tg{hf}",
                                         bufs=2, name=f"tg{s}_{hf}")
                            nc.scalar.activation(
                                tg[:], gx[ro:ro + 64, 1024 + co:1024 + co + sz],
                                mybir.ActivationFunctionType.Tanh)
                            si = lp.tile([B, sz], F32, tag=f"si{hf}",
                                         bufs=2, name=f"si{s}_{hf}")
                            nc.scalar.activation(
                                si[:], gx[ro:ro + 64, co:co + sz],
                                mybir.ActivationFunctionType.Sigmoid)
                            sf = lp.tile([B, sz], F32, tag=f"sf{hf}",
                                         bufs=2, name=f"sf{s}_{hf}")
                            nc.scalar.activation(
                                sf[:], gx[ro:ro + 64, 512 + co:512 + co + sz],
                                mybir.ActivationFunctionType.Sigmoid)
                            so = lp.tile([B, sz], F32, tag=f"so{hf}",
                                         bufs=2, name=f"so{s}_{hf}")
                            nc.scalar.activation(
                                so[:], gx[ro:ro + 64, 1536 + co:1536 + co + sz],
                                mybir.ActivationFunctionType.Sigmoid)
                            ig = lp.tile([B, sz], F32, tag=f"ig{hf}",
                                         bufs=2, name=f"ig{s}_{hf}")
                            nc.vector.tensor_mul(ig[:], si[:], tg[:])
                            fc = lp.tile([B, sz], F32, tag=f"fc{hf}",
                                         bufs=2, name=f"fc{s}_{hf}")
                            nc.vector.tensor_mul(fc[:], sf[:], cst[:, csl])
                            nc.vector.tensor_add(cst_new[:, csl], ig[:],
                                                 fc[:])
                            tcc = lp.tile([B, sz], F32, tag=f"tcc{hf}",
                                          bufs=2, name=f"tcc{s}_{hf}")
                            nc.scalar.activation(
                                tcc[:], cst_new[:, csl],
                                mybir.ActivationFunctionType.Tanh)
                            hh = lp.tile([B, sz], F32, tag=f"hh{hf}",
                                         bufs=2, name=f"hh{s}_{hf}")
                            nc.vector.tensor_mul(hh[:], so[:], tcc[:])
                            c0 = co // 128
                            for c4 in range(nch):
                                nc.tensor.transpose(
                                    tr_ps[:, (c0 + c4) * 64:
                                          (c0 + c4 + 1) * 64],
                                    hh[:, c4 * 128:(c4 + 1) * 128],
                                    ident[0:64, 0:64])
                            dst_hf = bass.AP(
                                tensor=hT_new[:].tensor,
                                offset=hT_new[:].offset + nhalf * 64
                                + c0 * 128,
                                ap=[hT_new[:].ap[0], [128, nch], [1, 64]])
                            nc.vector.tensor_copy(
                                dst_hf,
                                tr_ps[:, c0 * 64:(c0 + nch) * 64].rearrange(
                                    "p (a b) -> p a b", a=nch))
                        cst = cst_new
                        hT = hT_new
                        dst_h = bass.AP(tensor=hT[:].tensor,
                                        offset=hT[:].offset + nhalf * 64,
                                        ap=[hT[:].ap[0], [128, 4], [1, 64]])
                        weng = nc.sync if s % 2 == 0 else nc.scalar
                        weng.dma_start(
                            bass.AP(tensor=ht_dram.ap().tensor,
                                    offset=s * 128 * 256,
                                    ap=[[256, 128], [64, 4], [1, 64]]), dst_h)

            # ---------- emissions (half-projection) ----------
            with tc.tile_pool(name="emp", bufs=2) as ep, \
                 tc.tile_pool(name="psem", bufs=2, space="PSUM") as psem, \
                 tc.tile_pool(name="psemt", bufs=2, space="PSUM") as psemt:
                for tt in range(T // 8):
                    ht_in = ep.tile([128, 8 * 256], F32R, tag="ht_in", bufs=4,
                                    name=f"htin{tt}")
                    eng = (nc.sync, nc.scalar, nc.gpsimd)[tt % 3]
                    eng.dma_start(ht_in[:], bass.AP(
                        tensor=ht_dram.ap().tensor, offset=tt * 8 * 128 * 256,
                        ap=[[256, 128], [128 * 256, 8], [1, 256]]))
                    ht_rr = ep.tile([128, 8 * 256], F32R, tag="ht_rr", bufs=2,
                                    name=f"htrr{tt}")
                    nc.vector.tensor_copy(ht_rr[:], ht_in[:])
                    em_ps = psem.tile([K, 512], F32, tag="emps",
                                      name=f"emps{tt}")
                    for c in range(4):
                        rhs = bass.AP(tensor=ht_rr[:].tensor,
                                      offset=ht_rr[:].offset + c * 64,
                                      ap=[ht_rr[:].ap[0], [256, 8], [1, 64]])
                        nc.tensor.matmul(em_ps[:], wout_r[:, c, :], rhs,
                                         start=(c == 0), stop=(c == 3))
                    em_sb = ep.tile([K, 512], F32, tag="em_sb", bufs=2,
                                    name=f"emsb{tt}")
                    nc.vector.tensor_copy(em_sb[:], em_ps[:])
                    emt_ps = psemt.tile([128, 4 * K], F32, tag="emtps",
                                        name=f"emtps{tt}")
                    for tau in range(4):
                        nc.tensor.transpose(
                            emt_ps[:, tau * K:(tau + 1) * K],
                            em_sb[:, tau * 128:(tau + 1) * 128],
                            ident[0:K, 0:K])
                    emt_sb = ep.tile([128, 4 * K], F32, tag="emt_sb", bufs=2,
                                     name=f"emtsb{tt}")
                    nc.vector.tensor_tensor(
                        emt_sb[:].rearrange("p (a b) -> p a b", a=4),
                        emt_ps[:].rearrange("p (a b) -> p a b", a=4),
                        bass.AP(tensor=bout_sb[:].tensor,
                                offset=bout_sb[:].offset,
                                ap=[bout_sb[:].ap[0], [0, 4], [1, K]]),
                        op=mybir.AluOpType.add)
                    for tau in range(4):
                        nc.gpsimd.indirect_dma_start(
                            out=em_bounce.ap(),
                            out_offset=bass.IndirectOffsetOnAxis(
                                ap=emwidx_sb[:, tt * 4 + tau:tt * 4 + tau + 1],
                                axis=0),
                            in_=emt_sb[:, tau * K:(tau + 1) * K],
                            in_offset=None)

            nc.gpsimd.collective_compute(
                "AllReduce", mybir.AluOpType.add, replica_groups=groups,
                ins=[em_bounce.ap().opt()], outs=[em_shared.ap().opt()])

            # ---------- max-plus scan ----------
            with tc.tile_pool(name="scan", bufs=2) as sp:
                em_g = sp.tile([B, K], F32, tag="em_g", bufs=4, name="emg0")
                nc.gpsimd.indirect_dma_start(
                    out=em_g[:], out_offset=None, in_=em_shared.ap(),
                    in_offset=bass.IndirectOffsetOnAxis(
                        ap=emidx_sb[:, 0:1], axis=0))
                scur = sp.tile([B, K], F32, tag="scur", bufs=3, name="s0")
                nc.vector.tensor_add(scur[:], sinit_sb[:], em_g[:])
                nc.sync.dma_start(
                    bass.AP(tensor=scan_loc.ap().tensor, offset=0,
                            ap=[[K, B], [1, K]]), scur[:])
                trep3 = trep_sb[:].rearrange("p (a b) -> p a b", a=K)
                for k in range(1, T):
                    cand = sp.tile([B, K, K], F32, tag="cand", bufs=2,
                                   name=f"cand{k}")
                    sb_ap = bass.AP(tensor=scur[:].tensor,
                                    offset=scur[:].offset,
                                    ap=[scur[:].ap[0], [0, K], [1, K]])
                    nc.vector.tensor_tensor(cand[:], sb_ap, trep3,
                                            op=mybir.AluOpType.add)
                    nraw = sp.tile([B, K], F32, tag="nraw", bufs=2,
                                   name=f"nraw{k}")
                    nc.vector.tensor_reduce(nraw[:], cand[:],
                                            axis=mybir.AxisListType.X,
                                            op=mybir.AluOpType.max)
                    em_g = sp.tile([B, K], F32, tag="em_g", bufs=4,
                                   name=f"emg{k}")
                    nc.gpsimd.indirect_dma_start(
                        out=em_g[:], out_offset=None, in_=em_shared.ap(),
                        in_offset=bass.IndirectOffsetOnAxis(
                            ap=emidx_sb[:, k:k + 1], axis=0))
                    scur = sp.tile([B, K], F32, tag="scur", bufs=3,
                                   name=f"s{k}")
                    nc.vector.tensor_add(scur[:], nraw[:], em_g[:])
                    nc.sync.dma_start(
                        bass.AP(tensor=scan_loc.ap().tensor,
                                offset=k * B * K, ap=[[K, B], [1, K]]),
                        scur[:])

            nc.gpsimd.collective_compute(
                "AllGather", mybir.AluOpType.bypass, replica_groups=groups,
                ins=[scan_loc.ap().opt()], outs=[scan_gath.ap().opt()])

            # ---------- final argmax ----------
            TB = 32
            with tc.tile_pool(name="fin", bufs=2) as fp:
                for blk in range(T // TB):
                    t0 = blk * TB
                    k0 = (T - 1) - t0 - (TB - 1)
                    al = fp.tile([B, TB * K], F32, tag="al", bufs=2,
                                 name=f"al{blk}")
                    nc.sync.dma_start(al[:], bass.AP(
                        tensor=scan_gath.ap().tensor, offset=t0 * B * K,
                        ap=[[K, B], [B * K, TB], [1, K]]))
                    ga = fp.tile([B, TB * K], F32, tag="ga", bufs=2,
                                 name=f"ga{blk}")
                    nc.sync.dma_start(ga[:], bass.AP(
                        tensor=scan_gath.ap().tensor,
                        offset=(T * B + k0 * B) * K,
                        ap=[[K, B], [B * K, TB], [1, K]]))
                    emi = fp.tile([B, TB * K], F32, tag="emi", bufs=2,
                                  name=f"emi{blk}")
                    nc.sync.dma_start(emi[:], bass.AP(
                        tensor=em_shared.ap().tensor, offset=t0 * B * K,
                        ap=[[K, B], [B * K, TB], [1, K]]))
                    ga_rev = bass.AP(
                        tensor=ga[:].tensor,
                        offset=ga[:].offset + (TB - 1) * K,
                        ap=[ga[:].ap[0], [-K, TB], [1, K]])
                    tot = fp.tile([B, TB, K], F32, tag="tot", bufs=2,
                                  name=f"tot{blk}")
                    nc.vector.tensor_tensor(
                        tot[:], al[:].rearrange("p (a b) -> p a b", a=TB),
                        ga_rev, op=mybir.AluOpType.add)
                    nc.vector.tensor_tensor(
                        tot[:], tot[:],
                        emi[:].rearrange("p (a b) -> p a b", a=TB),
                        op=mybir.AluOpType.subtract)
                    mx = fp.tile([B, TB], F32, tag="mx", bufs=2,
                                 name=f"mx{blk}")
                    nc.vector.tensor_reduce(mx[:], tot[:],
                                            axis=mybir.AxisListType.X,
                                            op=mybir.AluOpType.max)
                    msk = fp.tile([B, TB, K], F32, tag="msk", bufs=2,
                                  name=f"msk{blk}")
                    nc.vector.tensor_tensor(
                        msk[:], tot[:],
                        bass.AP(tensor=mx[:].tensor, offset=mx[:].offset,
                                ap=[mx[:].ap[0], [1, TB], [0, K]]),
                        op=mybir.AluOpType.is_equal)
                    nc.vector.tensor_tensor(
                        msk[:], msk[:],
                        bass.AP(tensor=jshift[:].tensor,
                                offset=jshift[:].offset,
                                ap=[jshift[:].ap[0], [0, TB], [1, K]]),
                        op=mybir.AluOpType.mult)
                    jm = fp.tile([B, TB], F32, tag="jm", bufs=2,
                                 name=f"jm{blk}")
                    nc.vector.tensor_reduce(jm[:], msk[:],
                                            axis=mybir.AxisListType.X,
                                            op=mybir.AluOpType.min)
                    nc.vector.tensor_scalar_add(
                        tags_sb[:, t0:t0 + TB], jm[:], 1000.0)
            nc.sync.dma_start(tags_ap[:, :], tags_sb[:])

    nc.compile()
    return nc


def _host_prep(inputs, T=T_FULL):
    x = np.asarray(inputs["x"]).astype(np.int32)
    emb = np.ascontiguousarray(np.asarray(inputs["emb"], np.float32))
    TT = T * B
    NTILE = TT // 128
    t_map0 = np.arange(T)
    j = np.arange(NTILE)
    p = np.arange(128)
    t_loc = 2 * j[None, :] + (p[:, None] >= 64)
    zeros = {
        "idx": np.zeros((128, NTILE), np.int32),
        "emb": np.zeros((V, E), np.float32),
        "wih": np.zeros((E, G), np.float32),
        "bias": np.zeros((1, G), np.float32),
        "whh": np.zeros((H, G), np.float32),
        "wout": np.zeros((H, K), np.float32),
        "bout": np.zeros((128, K), np.float32),
        "trep": np.zeros((1, K * K), np.float32),
        "sinit": np.zeros((1, K), np.float32),
        "emidx": (t_map0[None, :] * B + np.arange(B)[:, None]).astype(np.int32),
        "emwidx": (t_map0[t_loc] * B + (p[:, None] % 64)).astype(np.int32),
    }
    maps = []
    for core in range(N_CORES):
        if core >= 2:
            maps.append(dict(zeros))
            continue
        d = "f" if core == 0 else "b"
        x_eff = x[:, :T] if core == 0 else np.ascontiguousarray(x[:, :T][:, ::-1])
        t_map = t_map0 if core == 0 else (T - 1) - t_map0
        tok = np.ascontiguousarray(x_eff.T).reshape(-1)
        m = {}
        m["idx"] = np.ascontiguousarray(tok.reshape(NTILE, 128).T)
        m["emb"] = emb
        m["wih"] = np.ascontiguousarray(
            np.asarray(inputs[f"w_ih_{d}"], np.float32).T)
        m["bias"] = np.asarray(inputs[f"b_{d}"], np.float32).reshape(1, G)
        m["whh"] = np.ascontiguousarray(
            np.asarray(inputs[f"w_hh_{d}"], np.float32).T)
        wo = np.asarray(inputs["w_out"], np.float32)
        half = wo[:, :H] if core == 0 else wo[:, H:]
        m["wout"] = np.ascontiguousarray(half.T)
        m["bout"] = (np.tile(np.asarray(inputs["b_out"], np.float32), (128, 1))
                     if core == 0 else np.zeros((128, K), np.float32))
        tr = np.asarray(inputs["crf_trans"], np.float32)
        m["trep"] = np.ascontiguousarray(
            (tr.T if core == 0 else tr).reshape(1, K * K))
        m["sinit"] = np.asarray(
            inputs["crf_start"] if core == 0 else inputs["crf_end"],
            np.float32).reshape(1, K)
        m["emidx"] = (t_map[None, :] * B
                      + np.arange(B)[:, None]).astype(np.int32)
        m["emwidx"] = (t_map[t_loc] * B + (p[:, None] % 64)).astype(np.int32)
        maps.append(m)
    return maps


_NC_CACHE = {}


def _get_nc(T=T_FULL):
    if T not in _NC_CACHE:
        _NC_CACHE[T] = _build_nc(T)
    return _NC_CACHE[T]


def kernel(**inputs):
    nc = _get_nc(T_FULL)
    maps = _host_prep(inputs, T_FULL)
    res = run_bass_kernel_spmd(nc, maps, core_ids=list(range(N_CORES)))
    return res.results[0]["tags"].astype(np.int32)

